# revision 1
# baseline (speedup 1.0000x reference)
"""NGU episodic-novelty kNN reward kernel for 8 Trainium2 NeuronCores.

Problem: for each of 64 envs, find the k=10 smallest squared distances
between obs[env] (256-d) and the first n_in_buffer[env] rows of its
8192-slot episode buffer, then compute the NGU novelty reward.

Strategy (memory-bound problem; ~512 MB of buffer data dominates):
  - Data-parallel over envs, 8 per core, but envs are assigned to
    (core, slot) by a snake distribution over descending n_in_buffer so
    that each slot's 8 envs (one per core) have similar buffer fill.
  - Slots beyond n_in_buffer can't affect the output (the reference
    masks them to BIG, and envs with n<k are zeroed), so the kernel
    only streams ceil(n_slotmax/2048) 2048-slot chunks per slot —
    roughly halving DMA for uniform n. Partially-valid chunks are
    pre-filled on host with MASK_FILL so masked slots get huge di.
  - Data is shipped as fp16 (halves DMA again). di errors ~1e-4
    relative; the final reward normalizes by the batch-average kth
    distance so correlated errors cancel further.
  - No on-device squaring: sum(d^2) per slot is precomputed on host
    (free CPU) and shipped as a tiny f32 side tensor.  TensorE computes
    2*dot with per-env block-diagonal 2*obs weights: 8 accumulating
    matmuls -> PSUM [4, 512] (PE can only write at partition base 0);
    VectorE fuses the PSUM read with the norm2 subtract, so each row
    holds -di + |obs|^2 (a per-env constant shift that preserves
    ordering; the host adds it back).  A tiny DMA scatters rows into
    the [128, 512] layout (skipped chunks keep the NEG_BIG memset).
  - VectorE max8 + match_replace + max8 -> per-row top-16 = the 16
    smallest di of each 512-slot group; DMA out cand [128, 16].
Host: per env, the union of its 16 groups' top-16 (256 values) is a
superset of the true top-k (k<=16); sort, take k, then run the tiny
cross-env normalization + reward epilogue in float32.
"""

import math

import numpy as np

CAP = 8192
NENV = 64
DIM = 256
NCORES = 8
EPV = NENV // NCORES      # env slots per core = 8
GROUPS = 16               # c-groups per env (512 slots each)
GSIZE = CAP // GROUPS     # 512
FCH = 4                   # max f-chunks per env (4 groups each)
M = 4                     # groups per matmul (output partitions)
DC = 8                    # d-chunks of 32
D32 = DIM // DC           # 32
P = 128
NEG_BIG = -3.0e38

EPS = 1e-3
MIN_DIST = 0.008
MAX_SIM = 2.0
L = 5.0

# input dtype config: "f32" or "f16"
DT_IN = "f16"
MASK_FILL = 1.0e9 if DT_IN == "f32" else 200.0

_PROGS = {}


def _np_in_dtype():
    return np.float32 if DT_IN == "f32" else np.float16


def _act_cost(n):
    return (224.0 + n) / 1.2


def _dve_sq_cost(n):
    if DT_IN == "f32":
        return (58.0 + n / 2.0) / 0.96 + (151.0 + n) / 0.96
    return (58.0 + n / 4.0) / 0.96 + (58.0 + n / 2.0) / 0.96


def _split_engines(trips):
    """Greedy ACT/DVE assignment per (slot, dc) tile; returns set of
    (slot, dc) handled by the vector engine."""
    dve_fixed = 25_000.0  # psum copies + top-k already on DVE (ns, rough)
    act_load, dve_load = 0.0, dve_fixed
    dve_tiles = set()
    for s, t in enumerate(trips):
        if t == 0:
            continue
        n = t * GSIZE
        for dc in range(DC):
            a, d = _act_cost(n), _dve_sq_cost(n)
            if dve_load + d < act_load + a:
                dve_load += d
                dve_tiles.add((s, dc))
            else:
                act_load += a
    return dve_tiles


def _build_program(trips, loop_n=None, knobs=None):
    from contextlib import ExitStack

    import concourse.bacc as bacc
    import concourse.mybir as mybir
    import concourse.tile as tile

    kn = {"bufs_loads": 5, "bufs_psums": 4, "bufs_cps": 2, "bufs_n2": 2,
          "ablate": None, "nq": 4, "scatter": "batch",
          "small_eng": "gpsimd", "load_eng": "sync"}
    kn.update(knobs or {})
    assert DT_IN == "f16"
    nq = kn["nq"]                  # dc's per load DMA
    nquad = DC // nq

    dt = mybir.dt
    dt_in = dt.float16

    tot = sum(trips)
    assert tot > 0
    offs = [0]
    for t in trips:
        offs.append(offs[-1] + t)

    # Bacc (not plain Bass): its compile() splits multi-sem waits into
    # event-semaphore instructions — the TRN2 ISA allows 1 wait per inst.
    nc = bacc.Bacc("TRN2", target_bir_lowering=False, num_devices=NCORES)
    dat = nc.dram_tensor("dat", [P, DC, tot, GSIZE], dt_in,
                         kind="ExternalInput")
    # per-env weights 2*obs on the block diagonal: [(g,d32), (s,dc,m)]
    w2 = nc.dram_tensor("w2", [P, EPV * DC * M], dt_in,
                        kind="ExternalInput")
    # host-precomputed sum(d^2) per buffer slot, chunk layout
    n2t = nc.dram_tensor("n2t", [tot, M, GSIZE], dt.float32,
                         kind="ExternalInput")
    cand = nc.dram_tensor("cand", [P, 16], dt.float32, kind="ExternalOutput")

    with ExitStack() as ctx:
        tc = ctx.enter_context(tile.TileContext(nc))
        consts = ctx.enter_context(tc.tile_pool(name="consts", bufs=1))
        loads = ctx.enter_context(tc.tile_pool(name="loads",
                                               bufs=kn["bufs_loads"]))
        psums = ctx.enter_context(tc.tile_pool(name="psums",
                                               bufs=kn["bufs_psums"],
                                               space="PSUM"))
        cps = ctx.enter_context(tc.tile_pool(name="cps", bufs=kn["bufs_cps"]))
        n2s = ctx.enter_context(tc.tile_pool(name="n2s", bufs=kn["bufs_n2"]))
        outp = ctx.enter_context(tc.tile_pool(name="outp", bufs=1))

        small = getattr(nc, kn["small_eng"])
        load_engs = [getattr(nc, e) for e in kn["load_eng"].split(",")]
        w_sb = consts.tile([P, EPV * DC * M], dt_in)
        small.dma_start(out=w_sb, in_=w2[:, :])

        def body():
            di_sb = outp.tile([P, GSIZE], dt.float32)  # -di, row=slot*16+grp
            nc.vector.memset(di_sb, NEG_BIG)

            for s in range(EPV):
                t_s = trips[s]
                if t_s == 0:
                    continue
                tq = []
                for q in range(nquad):
                    t = loads.tile([P, nq, FCH, GSIZE], dt_in, tag="t")
                    le = load_engs[(s * nquad + q) % len(load_engs)]
                    le.dma_start(
                        out=t[:, :, 0:t_s, :],
                        in_=dat[:, q * nq:(q + 1) * nq,
                                offs[s]:offs[s] + t_s, :])
                    tq.append(t)
                n2_sb = n2s.tile([M, FCH, GSIZE], dt.float32, tag="n2")
                small.dma_start(
                    out=n2_sb[:, 0:t_s, :],
                    in_=n2t[offs[s]:offs[s] + t_s].rearrange(
                        "f g j -> g f j"))
                if kn["ablate"] == "dmaonly":
                    continue
                cp = cps.tile([M, FCH, GSIZE], dt.float32, tag="cp")
                for f in range(t_s):
                    pt = psums.tile([M, GSIZE], dt.float32)
                    for dc in range(DC):
                        col = (s * DC + dc) * M
                        nc.tensor.matmul(
                            pt, w_sb[:, col:col + M],
                            tq[dc // nq][:, dc % nq, f, :],
                            start=(dc == 0), stop=(dc == DC - 1))
                    if kn["ablate"] == "nocp":
                        continue
                    # cp = 2*dot - n2 = -(di) + |obs|^2
                    nc.vector.tensor_sub(cp[:, f, :], pt, n2_sb[:, f, :])
                    if kn["scatter"] == "chunk":
                        row0 = s * GROUPS + f * M
                        small.dma_start(out=di_sb[row0:row0 + M, :],
                                        in_=cp[:, f, :])
                if kn["ablate"] == "nocp" or kn["scatter"] == "chunk":
                    continue
                row0 = s * GROUPS
                small.dma_start(
                    out=di_sb[row0:row0 + M * t_s, :].rearrange(
                        "(f g) j -> g f j", g=M),
                    in_=cp[:, 0:t_s, :])

            if kn["ablate"] == "notopk":
                return
            di_rep = outp.tile([P, GSIZE], dt.float32)
            cand_sb = outp.tile([P, 16], dt.float32)
            nc.vector.max(out=cand_sb[:, 0:8], in_=di_sb)
            nc.vector.match_replace(out=di_rep,
                                    in_to_replace=cand_sb[:, 0:8],
                                    in_values=di_sb, imm_value=NEG_BIG)
            nc.vector.max(out=cand_sb[:, 8:16], in_=di_rep)
            small.dma_start(out=cand[:, :], in_=cand_sb)

        if loop_n is None:
            body()
        else:
            with tc.For_i(0, loop_n, 1):
                body()

    nc.compile()
    return nc


def _get_program(trips, loop_n=None, knobs=None):
    key = (tuple(trips), loop_n, DT_IN,
           tuple(sorted((knobs or {}).items())))
    if key not in _PROGS:
        _PROGS[key] = _build_program(tuple(trips), loop_n, knobs)
    return _PROGS[key]


def _plan(n):
    """Snake-assign envs to (core, slot) by descending n; per-slot trip
    counts shared by all cores."""
    nn = np.clip(n, 0, CAP)
    order = np.argsort(-nn, kind="stable")
    env_of = np.empty((NCORES, EPV), np.int64)
    for s in range(EPV):
        idxs = order[s * NCORES:(s + 1) * NCORES]
        cores = range(NCORES) if s % 2 == 0 else range(NCORES - 1, -1, -1)
        for j, m in enumerate(cores):
            env_of[m, s] = idxs[j]
    trips = tuple(
        int(math.ceil(int(nn[order[s * NCORES]]) / (M * GSIZE)))
        for s in range(EPV))
    if sum(trips) == 0:
        trips = (1,) + trips[1:]
    return env_of, trips


def _make_in_maps(obs, data, n, env_of, trips):
    dt_np = _np_in_dtype()
    tot = sum(trips)
    offs = [0]
    for t in trips:
        offs.append(offs[-1] + t)

    data_masked = data.copy()
    for env in range(NENV):
        ne = int(min(max(n[env], 0), CAP))
        if ne < CAP:
            data_masked[ne:, env, :] = MASK_FILL

    in_maps = []
    for m in range(NCORES):
        dat_m = np.empty((P, DC, tot, GSIZE), dt_np)
        w2_m = np.zeros((P, EPV * DC * M), dt_np)
        n2_m = np.empty((tot, M, GSIZE), np.float32)
        for s in range(EPV):
            env = int(env_of[m, s])
            t_s = trips[s]
            o2 = (2.0 * obs[env]).reshape(DC, D32)     # [dc, d32]
            # w2[(g,d32), ((s,dc),m)] = 2*obs[env, dc*32+d32] if g==m
            for g in range(M):
                cols = (s * DC + np.arange(DC)) * M + g
                w2_m[g * D32:(g + 1) * D32, cols] = o2.T
            if t_s == 0:
                continue
            sub = data_masked[:t_s * M * GSIZE, env, :]     # [t*2048, 256]
            # c=(f*4+g)*512+j, d=dc*32+d32 -> [(g,d32), dc, f, j]
            dat_m[:, :, offs[s]:offs[s] + t_s, :] = (
                sub.reshape(t_s, M, GSIZE, DC, D32)
                   .transpose(1, 4, 3, 0, 2)
                   .reshape(P, DC, t_s, GSIZE))
            nrm = (sub.astype(np.float32) ** 2).sum(axis=1)  # [t*2048]
            n2_m[offs[s]:offs[s] + t_s] = nrm.reshape(t_s, M, GSIZE)
        in_maps.append({"dat": np.ascontiguousarray(dat_m),
                        "w2": w2_m, "n2t": n2_m})
    return in_maps


def _device_candidates(results, env_of, obs, k):
    """[NENV, k] ascending squared distances from per-core cand tensors.

    Device rows hold top-16 of (-di + |obs|^2); di = |obs|^2 - value."""
    o2 = (np.asarray(obs, np.float32) ** 2).sum(axis=1)       # [NENV]
    dists = np.empty((NENV, k), np.float32)
    for m in range(NCORES):
        c = np.asarray(results[m]["cand"], np.float32)        # [128, 16]
        for s in range(EPV):
            env = int(env_of[m, s])
            vals = o2[env] - c[s * GROUPS:(s + 1) * GROUPS, :].ravel()
            vals.sort()
            dists[env] = vals[:k]
    return dists


def _epilogue(dists, r_rnd, n, k):
    f32 = np.float32
    env_valid = n >= k
    dists = np.where(env_valid[:, None], dists, f32(0.0)).astype(np.float32)
    max_d = dists[:, -1]
    cnt = env_valid.sum()
    if cnt > 0:
        avg = f32(f32((max_d * env_valid).sum(dtype=np.float32))
                  / f32(max(cnt, 1)))
    else:
        avg = f32(0.0)
    denom = avg if avg > f32(1e-5) else f32(1.0)
    dists = (dists / denom).astype(np.float32)
    dists = np.maximum(dists - f32(MIN_DIST), f32(0.0))
    kern = (f32(EPS) / (dists + f32(EPS))).astype(np.float32)
    s = np.sqrt(f32(1.0) + kern.sum(axis=1, dtype=np.float32)).astype(np.float32)
    r = np.where(s > f32(MAX_SIM), f32(0.0), f32(1.0) / s).astype(np.float32)
    modifier = np.clip(np.asarray(r_rnd, np.float32), f32(1.0), f32(L))
    return (r * modifier).astype(np.float32)


def _run(obs, data, r_rnd, n_in_buffer, k, trace=False):
    from concourse.bass_utils import run_bass_kernel_spmd

    obs = np.asarray(obs, np.float32)
    data = np.asarray(data, np.float32)
    r_rnd = np.asarray(r_rnd, np.float32)
    n = np.asarray(n_in_buffer).astype(np.int64)
    k = int(k)
    assert k <= GROUPS, f"device top-16-per-group only covers k<=16, got {k}"

    env_of, trips = _plan(n)
    nc = _get_program(trips)
    in_maps = _make_in_maps(obs, data, n, env_of, trips)
    res = run_bass_kernel_spmd(nc, in_maps, list(range(NCORES)), trace=trace)
    dists = _device_candidates(res.results, env_of, obs, k)
    return _epilogue(dists, r_rnd, n, k), res


def kernel(obs, data, r_rnd, n_in_buffer, k):
    out, _ = _run(obs, data, r_rnd, n_in_buffer, k)
    return out



# revision 2
# speedup vs baseline: 2.1902x; 2.1902x over previous
"""NGU episodic-novelty kNN reward kernel for 8 Trainium2 NeuronCores.

Problem: for each of 64 envs, find the k=10 smallest squared distances
between obs[env] (256-d) and the first n_in_buffer[env] rows of its
8192-slot episode buffer, then compute the NGU novelty reward.

Strategy (memory-bound; streaming the buffer from HBM dominates):
  - Data-parallel over envs, 8 per core; envs are assigned to
    (core, slot) by a snake distribution over descending n_in_buffer so
    each slot's 8 envs (one per core) have similar buffer fill.
  - Data ships as fp8 e4m3 (quarter of f32 DMA). The per-slot norm2 is
    precomputed on host in f32 FROM THE QUANTIZED values, so the device
    computes exact distances to the quantized points (plus a per-env
    constant |obs|^2-|obs8|^2 shift that preserves ordering); measured
    end-to-end rel err ~2e-5.
  - Exact-length streaming: slots are interleaved slot = j*16 + g over
    G=16 groups; the j extent Je = ceil(n_slotmax/16) is exact, so no
    2048-slot chunk rounding. Invalid slots inside the streamed prefix
    are masked by norm2 = +1e9 (no data masking needed); j >= Je is
    never touched by any matmul so garbage can't flow.
  - TensorE: per env, 16 accumulating DoubleRow fp8 matmuls (each
    contracts 16 dims: 8 partitions x 2 rows) with per-env
    block-diagonal 2*obs weights -> PSUM [16, Je] holding 2*dot.
    DoubleRow processes 2 fp8 rows/cycle (0.5 cycles per output col).
  - VectorE fuses the PSUM read with the norm2 subtract: cp = 2*dot -
    norm2 = -di + |obs|^2. A small DMA scatters cp into the [128, 512]
    di layout (row = env*16 + g); unwritten cols keep the NEG_BIG
    memset.
  - VectorE max8 + match_replace + max8 -> per-row top-16 = the 16
    smallest di of each group; DMA out cand [128, 16].
Host: per env, the union of its 16 groups' top-16 (256 values) is a
superset of the true top-k (k<=16); sort, take k, then run the tiny
cross-env normalization + reward epilogue in float32.
"""

import math

import numpy as np

CAP = 8192
NENV = 64
DIM = 256
NCORES = 8
EPV = NENV // NCORES      # env slots per core = 8
G = 16                    # groups per env (slot = j*16 + g)
J = CAP // G              # 512 max j extent
DC = 16                   # DoubleRow matmul steps per env
T = 2                     # contraction rows per partition (DoubleRow)
D8 = 8                    # dims per (step, t) = partitions per group
P = 128
NEG_BIG = -3.0e38
N2_MASK = 1.0e9           # norm2 value for invalid slots

EPS = 1e-3
MIN_DIST = 0.008
MAX_SIM = 2.0
L = 5.0

_PROGS = {}


def _np_fp8():
    import ml_dtypes

    return ml_dtypes.float8_e4m3


def _build_program(jext, loop_n=None, knobs=None):
    from contextlib import ExitStack

    import concourse.bacc as bacc
    import concourse.mybir as mybir
    import concourse.tile as tile

    kn = {"bufs_loads": 3, "bufs_psums": 8, "bufs_cps": 2, "bufs_n2": 2,
          "ablate": None, "small_eng": "gpsimd", "load_eng": "sync",
          "scat_eng": "scalar", "layout": "djt"}
    kn.update(knobs or {})

    dt = mybir.dt
    dt8 = dt.float8e4

    assert len(jext) == EPV and max(jext) <= J
    assert sum(jext) > 0

    nc = bacc.Bacc("TRN2", target_bir_lowering=False, num_devices=NCORES)
    if kn["layout"] == "djt":
        dat = nc.dram_tensor("dat", [P, EPV, DC, J, T], dt8,
                             kind="ExternalInput")
    else:  # jdt
        dat = nc.dram_tensor("dat", [P, EPV, J, DC, T], dt8,
                             kind="ExternalInput")
    # per-env block-diag weights 2*obs: [(g,d8), (s, dc, t, m)]
    w2 = nc.dram_tensor("w2", [P, EPV * DC * T * G], dt8,
                        kind="ExternalInput")
    # host-precomputed sum(d^2) of quantized slots: [g, s, j]
    n2t = nc.dram_tensor("n2t", [G, EPV, J], dt.float32,
                         kind="ExternalInput")
    cand = nc.dram_tensor("cand", [P, 16], dt.float32, kind="ExternalOutput")

    with ExitStack() as ctx:
        tc = ctx.enter_context(tile.TileContext(nc))
        consts = ctx.enter_context(tc.tile_pool(name="consts", bufs=1))
        loads = ctx.enter_context(tc.tile_pool(name="loads",
                                               bufs=kn["bufs_loads"]))
        psums = ctx.enter_context(tc.tile_pool(name="psums",
                                               bufs=kn["bufs_psums"],
                                               space="PSUM"))
        cps = ctx.enter_context(tc.tile_pool(name="cps", bufs=kn["bufs_cps"]))
        n2s = ctx.enter_context(tc.tile_pool(name="n2s", bufs=kn["bufs_n2"]))
        outp = ctx.enter_context(tc.tile_pool(name="outp", bufs=1))

        small = getattr(nc, kn["small_eng"])
        scat = getattr(nc, kn["scat_eng"])
        load_engs = [getattr(nc, e) for e in kn["load_eng"].split(",")]
        w_sb = consts.tile([P, EPV, DC, T, G], dt8)
        small.dma_start(out=w_sb,
                        in_=w2.rearrange("p (s c t m) -> p s c t m",
                                         s=EPV, c=DC, t=T))

        def body():
            di_sb = outp.tile([P, J], dt.float32)  # cp, row = s*16 + g
            nc.vector.memset(di_sb, NEG_BIG)

            for s in range(EPV):
                je = jext[s]
                if je == 0:
                    continue
                if kn["layout"] == "djt":
                    t_sb = loads.tile([P, DC, J, T], dt8, tag="t")
                    le = load_engs[s % len(load_engs)]
                    le.dma_start(out=t_sb[:, :, 0:je, :],
                                 in_=dat[:, s, :, 0:je, :])
                else:
                    t_sb = loads.tile([P, J, DC, T], dt8, tag="t")
                    le = load_engs[s % len(load_engs)]
                    le.dma_start(out=t_sb[:, 0:je, :, :],
                                 in_=dat[:, s, 0:je, :, :])
                n2_sb = n2s.tile([G, J], dt.float32, tag="n2")
                small.dma_start(out=n2_sb[:, 0:je], in_=n2t[:, s, 0:je])
                if kn["ablate"] == "dmaonly":
                    continue
                pt = psums.tile([G, J], dt.float32)
                for c in range(DC):
                    if kn["layout"] == "djt":
                        rhs = t_sb[:, c, 0:je, :].rearrange("p j t -> p t j")
                    else:
                        rhs = t_sb[:, 0:je, c, :].rearrange("p j t -> p t j")
                    nc.tensor.matmul(
                        pt[:, 0:je], w_sb[:, s, c, :, :], rhs,
                        start=(c == 0), stop=(c == DC - 1),
                        perf_mode=mybir.MatmulPerfMode.DoubleRow)
                if kn["ablate"] == "nocp":
                    continue
                # cp = 2*dot - n2 = -(di) + |obs|^2
                cp = cps.tile([G, J], dt.float32, tag="cp")
                nc.vector.tensor_sub(cp[:, 0:je], pt[:, 0:je], n2_sb[:, 0:je])
                scat.dma_start(out=di_sb[s * G:(s + 1) * G, 0:je],
                               in_=cp[:, 0:je])

            if kn["ablate"] in ("notopk", "nocp", "dmaonly"):
                return
            di_rep = outp.tile([P, J], dt.float32)
            cand_sb = outp.tile([P, 16], dt.float32)
            nc.vector.max(out=cand_sb[:, 0:8], in_=di_sb)
            nc.vector.match_replace(out=di_rep,
                                    in_to_replace=cand_sb[:, 0:8],
                                    in_values=di_sb, imm_value=NEG_BIG)
            nc.vector.max(out=cand_sb[:, 8:16], in_=di_rep)
            small.dma_start(out=cand[:, :], in_=cand_sb)

        if loop_n is None:
            body()
        else:
            with tc.For_i(0, loop_n, 1):
                body()

    nc.compile()
    return nc


def _get_program(jext, loop_n=None, knobs=None):
    key = (tuple(jext), loop_n,
           tuple(sorted((knobs or {}).items())))
    if key not in _PROGS:
        _PROGS[key] = _build_program(tuple(jext), loop_n, knobs)
    return _PROGS[key]


def _plan(n):
    """Snake-assign envs to (core, slot) by descending n; per-slot j
    extents (ceil(max_n/16)) shared by all cores."""
    nn = np.clip(n, 0, CAP)
    order = np.argsort(-nn, kind="stable")
    env_of = np.empty((NCORES, EPV), np.int64)
    for s in range(EPV):
        idxs = order[s * NCORES:(s + 1) * NCORES]
        cores = range(NCORES) if s % 2 == 0 else range(NCORES - 1, -1, -1)
        for j, m in enumerate(cores):
            env_of[m, s] = idxs[j]
    jext = tuple(
        int(math.ceil(int(nn[order[s * NCORES]]) / G))
        for s in range(EPV))
    if sum(jext) == 0:
        jext = (1,) + jext[1:]
    return env_of, jext


def _make_in_maps(obs, data, n, env_of, jext, layout="djt"):
    dt8 = _np_fp8()
    data8 = np.asarray(data, np.float32).astype(dt8)       # [C, N, D]
    data8f = data8.astype(np.float32)
    n2_full = (data8f ** 2).sum(axis=-1)                   # [C, N]
    # mask invalid slots
    slot_idx = np.arange(CAP)[:, None]
    n2_full = np.where(slot_idx < n[None, :], n2_full, N2_MASK)
    obs8 = (2.0 * np.asarray(obs, np.float32)).astype(dt8)  # [N, D]
    obs8f = obs8.astype(np.float32)

    in_maps = []
    for m in range(NCORES):
        if layout == "djt":
            dat_m = np.zeros((P, EPV, DC, J, T), dt8)
        else:
            dat_m = np.zeros((P, EPV, J, DC, T), dt8)
        w2_m = np.zeros((P, EPV, DC, T, G), dt8)
        n2_m = np.full((G, EPV, J), np.float32(N2_MASK), np.float32)
        for s in range(EPV):
            env = int(env_of[m, s])
            # d = dc*16 + t*8 + d8 ; slot c = j*16 + g ; p = g*8 + d8
            x = data8[:, env, :].reshape(J, G, DC, T, D8)  # [j, g, dc, t, d8]
            xt = x.transpose(1, 4, 2, 0, 3)                # [g, d8, dc, j, t]
            if layout == "djt":
                dat_m[:, s] = xt.reshape(P, DC, J, T)
            else:
                dat_m[:, s] = xt.transpose(0, 1, 3, 2, 4).reshape(P, J, DC, T)
            o = obs8[env].reshape(DC, T, D8)               # [dc, t, d8]
            for g in range(G):
                w2_m[g * D8:(g + 1) * D8, s, :, :, g] = o.transpose(2, 0, 1)
            n2_m[:, s, :] = n2_full[:, env].reshape(J, G).T
        in_maps.append({"dat": np.ascontiguousarray(dat_m),
                        "w2": np.ascontiguousarray(
                            w2_m.reshape(P, EPV * DC * T * G)),
                        "n2t": n2_m})
    return in_maps


def _device_candidates(results, env_of, obs, k):
    """[NENV, k] ascending squared distances from per-core cand tensors.

    Device rows hold top-16 of (-di + |obs|^2); di = |obs|^2 - value."""
    o2 = (np.asarray(obs, np.float32) ** 2).sum(axis=1)       # [NENV]
    dists = np.empty((NENV, k), np.float32)
    for m in range(NCORES):
        c = np.asarray(results[m]["cand"], np.float32)        # [128, 16]
        for s in range(EPV):
            env = int(env_of[m, s])
            vals = o2[env] - c[s * G:(s + 1) * G, :].ravel()
            vals.sort()
            dists[env] = vals[:k]
    return dists


def _epilogue(dists, r_rnd, n, k):
    f32 = np.float32
    env_valid = n >= k
    dists = np.where(env_valid[:, None], dists, f32(0.0)).astype(np.float32)
    max_d = dists[:, -1]
    cnt = env_valid.sum()
    if cnt > 0:
        avg = f32(f32((max_d * env_valid).sum(dtype=np.float32))
                  / f32(max(cnt, 1)))
    else:
        avg = f32(0.0)
    denom = avg if avg > f32(1e-5) else f32(1.0)
    dists = (dists / denom).astype(np.float32)
    dists = np.maximum(dists - f32(MIN_DIST), f32(0.0))
    kern = (f32(EPS) / (dists + f32(EPS))).astype(np.float32)
    s = np.sqrt(f32(1.0) + kern.sum(axis=1, dtype=np.float32)).astype(np.float32)
    r = np.where(s > f32(MAX_SIM), f32(0.0), f32(1.0) / s).astype(np.float32)
    modifier = np.clip(np.asarray(r_rnd, np.float32), f32(1.0), f32(L))
    return (r * modifier).astype(np.float32)


def _run(obs, data, r_rnd, n_in_buffer, k, trace=False, knobs=None):
    from concourse.bass_utils import run_bass_kernel_spmd

    obs = np.asarray(obs, np.float32)
    data = np.asarray(data, np.float32)
    r_rnd = np.asarray(r_rnd, np.float32)
    n = np.asarray(n_in_buffer).astype(np.int64)
    k = int(k)
    assert k <= 16, f"device top-16-per-group only covers k<=16, got {k}"

    env_of, jext = _plan(n)
    nc = _get_program(jext, knobs=knobs)
    layout = (knobs or {}).get("layout", "djt")
    in_maps = _make_in_maps(obs, data, n, env_of, jext, layout=layout)
    res = run_bass_kernel_spmd(nc, in_maps, list(range(NCORES)), trace=trace)
    dists = _device_candidates(res.results, env_of, obs, k)
    return _epilogue(dists, r_rnd, n, k), res


def kernel(obs, data, r_rnd, n_in_buffer, k):
    out, _ = _run(obs, data, r_rnd, n_in_buffer, k)
    return out


# revision 22
# speedup vs baseline: 3.1860x; 1.4547x over previous
"""NGU episodic-novelty kNN reward kernel for 8 Trainium2 NeuronCores.

Problem: for each of 64 envs, find the k=10 smallest squared distances
between obs[env] (256-d) and the first n_in_buffer[env] rows of its
8192-slot episode buffer, then compute the NGU novelty reward.

Strategy (memory-bound; streaming the buffer from HBM dominates):
  - Data ships as fp8 e4m3 (quarter of f32 DMA). The per-slot norm2 is
    precomputed on host in f32 FROM THE QUANTIZED values, so the device
    computes exact distances to the quantized points (plus a per-env
    constant |obs|^2-|obs8|^2 shift that preserves ordering); measured
    end-to-end rel err ~1e-5.
  - Work unit is a BIN: 16 bins (= PSUM partitions) per slot-position,
    8 slot-positions per core -> 1024 bins across the 8 cores. A bin
    holds a contiguous range of ONE env's buffer slots (<= caps[s]
    slots, caps a multiple of 64). The host bin-packs envs into bins
    (greedy over capacity vectors) so the total streamed columns
    approach sum(n)/128 -- no 2048-slot chunk rounding and no
    max-across-cores waste. Slack inside a bin is masked via
    norm2 = +1e9 (no data masking); columns past caps are never
    touched by any matmul, so garbage can't flow.
  - TensorE: per slot-position, 16 accumulating DoubleRow fp8 matmuls
    (each contracts 16 dims: 8 partitions x 2 rows) with block-diagonal
    2*obs weights (each bin's own env) -> PSUM [16, caps] holding
    2*dot. DoubleRow processes 2 fp8 rows/cycle.
  - Data layout [p][jblock][dc][t][j64]: DMA descriptors are 2KB
    contiguous runs while each DoubleRow XBUS stream walks stride-1
    bytes inside a 64-wide block (strided ifmap reads are ~30x slower).
  - VectorE per slot-position: cp = PSUM - norm2 (the only PSUM
    reader), then a fused per-bin top-16: max8 + match_replace + max8
    straight out of cp into cand[16, s*16:s*16+16]. No [128, 512]
    scatter, no end-of-body global top-k: the tail after the last DMA
    lands is one tiny sub+topk (64 cols) and the cand store.
Host: per env, the union of its bins' top-16s is a superset of the
true top-k (k<=16 per bin); sort, take k, then run the tiny cross-env
normalization + reward epilogue in float32.
"""

import math

import numpy as np

CAP = 8192
NENV = 64
DIM = 256
NCORES = 8
EPV = 8                   # slot-positions per core
G = 16                    # bins (groups) per slot-position
NBINS = G * EPV           # 128 bins per core
J = 512                   # max columns per bin
DC = 16                   # DoubleRow matmul steps per slot-position
T = 2                     # contraction rows per partition (DoubleRow)
D8 = 8                    # dims per (step, t) = partitions per bin
P = 128
NEG_BIG = -3.0e38
N2_MASK = 1.0e9           # norm2 value for invalid/slack columns

EPS = 1e-3
MIN_DIST = 0.008
MAX_SIM = 2.0
L = 5.0

_PROGS = {}
_PLANS = {}


def _np_fp8():
    import ml_dtypes

    return ml_dtypes.float8_e4m3


def _build_program(caps, loop_n=None, knobs=None):
    from contextlib import ExitStack

    import concourse.bacc as bacc
    import concourse.mybir as mybir
    import concourse.tile as tile

    kn = {"bufs_loads": 4, "bufs_psums": 8, "bufs_cps": 3, "bufs_n2": 6,
          "bufs_out": 2, "unroll": 1,
          "ablate": None, "small_eng": "gpsimd", "load_eng": "sync",
          "out_eng": "scalar"}
    kn.update(knobs or {})

    dt = mybir.dt
    dt8 = dt.float8e4

    assert len(caps) == EPV and max(caps) <= J
    assert all(c % 64 == 0 for c in caps)
    jbext = [c // 64 for c in caps]
    totjb = sum(jbext)
    boffs = [0]
    for jb in jbext:
        boffs.append(boffs[-1] + jb)

    nc = bacc.Bacc("TRN2", target_bir_lowering=False, num_devices=NCORES)
    dat = nc.dram_tensor("dat", [P, totjb, DC, T, 64], dt8,
                         kind="ExternalInput")
    # block-diag weights 2*obs (per-bin env): [(g,d8), (s, dc, t, m)]
    w2 = nc.dram_tensor("w2", [P, EPV * DC * T * G], dt8,
                        kind="ExternalInput")
    # host-precomputed sum(d^2) of quantized slots, +1e9 on slack: [g, s, j]
    n2t = nc.dram_tensor("n2t", [G, EPV, J], dt.float32,
                         kind="ExternalInput")
    cand = nc.dram_tensor("cand", [G, EPV * 16], dt.float32,
                          kind="ExternalOutput")

    with ExitStack() as ctx:
        tc = ctx.enter_context(tile.TileContext(nc))
        consts = ctx.enter_context(tc.tile_pool(name="consts", bufs=1))
        loads = ctx.enter_context(tc.tile_pool(name="loads",
                                               bufs=kn["bufs_loads"]))
        psums = ctx.enter_context(tc.tile_pool(name="psums",
                                               bufs=kn["bufs_psums"],
                                               space="PSUM"))
        cps = ctx.enter_context(tc.tile_pool(name="cps", bufs=kn["bufs_cps"]))
        n2s = ctx.enter_context(tc.tile_pool(name="n2s", bufs=kn["bufs_n2"]))
        outp = ctx.enter_context(tc.tile_pool(name="outp",
                                              bufs=kn["bufs_out"]))

        small = getattr(nc, kn["small_eng"])
        oute = getattr(nc, kn["out_eng"])
        load_engs = [getattr(nc, e) for e in kn["load_eng"].split(",")]
        w_sb = consts.tile([P, EPV, DC, T, G], dt8)
        small.dma_start(out=w_sb,
                        in_=w2.rearrange("p (s c t m) -> p s c t m",
                                         s=EPV, c=DC, t=T))

        def body():
            cand_sb = outp.tile([G, EPV * 16], dt.float32)
            for s in range(EPV):
                je = caps[s]
                jb = jbext[s]
                if jb == 0:
                    continue
                le = load_engs[s % len(load_engs)]
                t_sb = loads.tile([P, J // 64, DC, T, 64], dt8, tag="t")
                le.dma_start(out=t_sb[:, 0:jb, :, :, :],
                             in_=dat[:, boffs[s]:boffs[s] + jb, :, :, :])
                n2_sb = n2s.tile([G, J], dt.float32, tag="n2")
                small.dma_start(out=n2_sb[:, 0:je], in_=n2t[:, s, 0:je])
                if kn["ablate"] == "dmaonly":
                    continue
                pt = psums.tile([G, J], dt.float32)
                for c in range(DC):
                    rhs = t_sb[:, 0:jb, c, :, :].rearrange(
                        "p b t j -> p t b j")
                    nc.tensor.matmul(
                        pt[:, 0:je], w_sb[:, s, c, :, :], rhs,
                        start=(c == 0), stop=(c == DC - 1),
                        perf_mode=mybir.MatmulPerfMode.DoubleRow)
                if kn["ablate"] == "nocp":
                    continue
                # cp = 2*dot - n2 = -(di) + |obs|^2
                cp = cps.tile([G, J], dt.float32, tag="cp")
                nc.vector.tensor_sub(cp[:, 0:je], pt[:, 0:je], n2_sb[:, 0:je])
                if kn["ablate"] == "notopk":
                    continue
                col = s * 16
                nc.vector.max(out=cand_sb[:, col:col + 8], in_=cp[:, 0:je])
                rep = cps.tile([G, J], dt.float32, tag="rep")
                nc.vector.match_replace(out=rep[:, 0:je],
                                        in_to_replace=cand_sb[:, col:col + 8],
                                        in_values=cp[:, 0:je],
                                        imm_value=NEG_BIG)
                nc.vector.max(out=cand_sb[:, col + 8:col + 16],
                              in_=rep[:, 0:je])

            if kn["ablate"] is None:
                oute.dma_start(out=cand[:, :], in_=cand_sb)

        if loop_n is None:
            body()
        else:
            # For_i ends every iteration with an all-engine barrier
            # (semaphore reset), so iterations cannot pipeline. Emitting
            # `unroll` bodies per iteration lets bodies overlap through
            # the tile pools and amortizes the barrier + pipeline drain.
            with tc.For_i(0, loop_n, 1):
                for _ in range(kn["unroll"]):
                    body()

    nc.compile()
    return nc


def _get_program(caps, loop_n=None, knobs=None):
    key = (tuple(caps), loop_n,
           tuple(sorted((knobs or {}).items())))
    if key not in _PROGS:
        _PROGS[key] = _build_program(tuple(caps), loop_n, knobs)
    return _PROGS[key]


def _pack(caps, nn):
    """Greedy: envs desc; fill with the largest caps that fit exactly;
    remainder -> least-waste free bin. Returns per-class bin lists
    [[(env, c0, L), ...] x 8] or None if infeasible."""
    free = [G * NCORES] * EPV  # 128 bins per slot-class (16 x 8 cores)
    cls = [[] for _ in range(EPV)]
    order = sorted(range(EPV), key=lambda s: -caps[s])
    for e in np.argsort(-nn, kind="stable"):
        rem = int(nn[e])
        c0 = 0
        while rem > 0:
            pick = None
            for s in order:
                if caps[s] and caps[s] <= rem and free[s] > 0:
                    pick = s
                    break
            if pick is None:
                cands = [s for s in range(EPV)
                         if free[s] > 0 and caps[s] >= rem]
                if not cands:
                    return None
                pick = min(cands, key=lambda s: caps[s])
            length = min(rem, caps[pick])
            cls[pick].append((int(e), c0, length))
            free[pick] -= 1
            rem -= length
            c0 += length
    return cls


def _plan(n):
    """Bin-pack envs into the 1024 (core, slot, group) bins.

    Returns (bins, caps): caps[s] = column capacity of slot-position s
    (multiple of 64, shared by all cores); bins[m][s][g] = (env, c0, L)
    or None."""
    key = tuple(np.asarray(n).tolist())
    if key in _PLANS:
        return _PLANS[key]
    from itertools import combinations_with_replacement

    nn = np.clip(np.asarray(n), 0, CAP).astype(np.int64)
    best = None
    if nn.sum() > 0:
        for caps in combinations_with_replacement(
                (512, 448, 384, 320, 256, 192, 128, 64), EPV):
            if best is not None and sum(caps) >= best[0]:
                continue
            cls = _pack(caps, nn)
            if cls is not None:
                best = (sum(caps), caps, cls)
    if best is None:
        caps, cls = (64,) * EPV, [[] for _ in range(EPV)]
    else:
        _, caps, cls = best
        caps = tuple(sorted(caps, reverse=True))
        # re-pack against the sorted caps so class indices line up
        cls = _pack(caps, nn)
    bins = [[[None] * G for _ in range(EPV)] for _ in range(NCORES)]
    for s in range(EPV):
        for i, piece in enumerate(cls[s]):
            m, g = i // G, i % G
            bins[m][s][g] = piece
    _PLANS[key] = (bins, caps)
    return bins, caps


def _make_in_maps(obs, data, n, bins, caps):
    dt8 = _np_fp8()
    data8 = np.asarray(data, np.float32).astype(dt8)       # [C, N, D]
    n2_full = (data8.astype(np.float32) ** 2).sum(axis=-1)  # [C, N]
    slot_idx = np.arange(CAP)[:, None]
    n2_full = np.where(slot_idx < n[None, :], n2_full, N2_MASK)
    obs8 = (2.0 * np.asarray(obs, np.float32)).astype(dt8)  # [N, D]

    jbext = [c // 64 for c in caps]
    totjb = sum(jbext)
    boffs = [0]
    for jb in jbext:
        boffs.append(boffs[-1] + jb)

    in_maps = []
    for m in range(NCORES):
        dat_m = np.zeros((P, totjb, DC, T, 64), dt8)
        w2_m = np.zeros((P, EPV, DC, T, G), dt8)
        n2_m = np.full((G, EPV, J), np.float32(N2_MASK), np.float32)
        for s in range(EPV):
            cap_s = caps[s]
            for g in range(G):
                piece = bins[m][s][g]
                if piece is None:
                    continue
                e, c0, length = piece
                # col j of bin = slot c0+j ; j = jb*64 + j64 ;
                # d = dc*16 + t*8 + d8 ; partition = g*8 + d8
                x = np.zeros((cap_s, DIM), dt8)
                x[:length] = data8[c0:c0 + length, e, :]
                xt = (x.reshape(cap_s // 64, 64, DC, T, D8)
                       .transpose(4, 0, 2, 3, 1))   # [d8, jb, dc, t, 64]
                dat_m[g * D8:(g + 1) * D8,
                      boffs[s]:boffs[s] + cap_s // 64] = xt
                o = obs8[e].reshape(DC, T, D8)      # [dc, t, d8]
                w2_m[g * D8:(g + 1) * D8, s, :, :, g] = o.transpose(2, 0, 1)
                n2_m[g, s, 0:length] = n2_full[c0:c0 + length, e]
        in_maps.append({"dat": np.ascontiguousarray(dat_m),
                        "w2": np.ascontiguousarray(
                            w2_m.reshape(P, EPV * DC * T * G)),
                        "n2t": n2_m})
    return in_maps


def _device_candidates(results, bins, obs, k):
    """[NENV, k] ascending squared distances from per-core cand tensors.

    cand[g, s*16:(s+1)*16] holds the top-16 of (-di + |obs|^2) for bin
    (m, s, g); di = |obs|^2 - value."""
    o2 = (np.asarray(obs, np.float32) ** 2).sum(axis=1)       # [NENV]
    vals = [[] for _ in range(NENV)]
    for m in range(NCORES):
        c = np.asarray(results[m]["cand"], np.float32)        # [16, 128]
        for s in range(EPV):
            for g in range(G):
                piece = bins[m][s][g]
                if piece is None:
                    continue
                e = piece[0]
                vals[e].append(o2[e] - c[g, s * 16:(s + 1) * 16])
    dists = np.zeros((NENV, k), np.float32)
    for e in range(NENV):
        if not vals[e]:
            continue
        v = np.concatenate(vals[e])
        v.sort()
        if v.size >= k:
            dists[e] = v[:k]
        else:
            dists[e, :v.size] = v
            dists[e, v.size:] = v[-1] if v.size else 0.0
    return dists


def _epilogue(dists, r_rnd, n, k):
    f32 = np.float32
    env_valid = n >= k
    dists = np.where(env_valid[:, None], dists, f32(0.0)).astype(np.float32)
    max_d = dists[:, -1]
    cnt = env_valid.sum()
    if cnt > 0:
        avg = f32(f32((max_d * env_valid).sum(dtype=np.float32))
                  / f32(max(cnt, 1)))
    else:
        avg = f32(0.0)
    denom = avg if avg > f32(1e-5) else f32(1.0)
    dists = (dists / denom).astype(np.float32)
    dists = np.maximum(dists - f32(MIN_DIST), f32(0.0))
    kern = (f32(EPS) / (dists + f32(EPS))).astype(np.float32)
    s = np.sqrt(f32(1.0) + kern.sum(axis=1, dtype=np.float32)).astype(np.float32)
    r = np.where(s > f32(MAX_SIM), f32(0.0), f32(1.0) / s).astype(np.float32)
    modifier = np.clip(np.asarray(r_rnd, np.float32), f32(1.0), f32(L))
    return (r * modifier).astype(np.float32)


def _run(obs, data, r_rnd, n_in_buffer, k, trace=False, knobs=None):
    from concourse.bass_utils import run_bass_kernel_spmd

    obs = np.asarray(obs, np.float32)
    data = np.asarray(data, np.float32)
    r_rnd = np.asarray(r_rnd, np.float32)
    n = np.asarray(n_in_buffer).astype(np.int64)
    k = int(k)
    if k > 16:  # device top-16-per-bin only covers k<=16
        o2 = (obs ** 2).sum(axis=1)
        dot = np.einsum("nd,cnd->nc", obs, data, dtype=np.float32)
        n2 = (data.astype(np.float32) ** 2).sum(axis=-1)
        di = o2[:, None] + n2.T - 2.0 * dot
        di = np.where(np.arange(CAP)[None, :] < n[:, None], di, 1e30)
        dists = np.sort(di, axis=1)[:, :k].astype(np.float32)
        return _epilogue(dists, r_rnd, n, k), None

    bins, caps = _plan(n)
    nc = _get_program(caps, knobs=knobs)
    in_maps = _make_in_maps(obs, data, n, bins, caps)
    res = run_bass_kernel_spmd(nc, in_maps, list(range(NCORES)), trace=trace)
    dists = _device_candidates(res.results, bins, obs, k)
    return _epilogue(dists, r_rnd, n, k), res


def kernel(obs, data, r_rnd, n_in_buffer, k):
    out, _ = _run(obs, data, r_rnd, n_in_buffer, k)
    return out
